# revision 5
# baseline (speedup 1.0000x reference)
"""Trainium2 Bass kernel for nn_GroupedMultiheadSelfAttention.

Sharding: data-parallel over batch — 8 batch elements, one per NeuronCore.
Each core computes all 16 (a,b) attention pairs for its batch element.

Math (biases are structurally zero in this problem):
  For pair (a,b):  Q = x_a @ Wq, K = x_b @ Wk, V = x_b @ Wv
                   E = exp(Q K^T / sqrt(inner)),  w_ab = rowsum(E)
  Reference output per group a:
      out_a = sum_b softmax(L_ab) @ V_b @ Wo_ab * (w_ab / sum_b w_ab)
            = (sum_b E_ab @ V_b @ Wo_ab) / (sum_b w_ab)
  so no per-pair normalization is needed: accumulate E@V@Wo in PSUM across b
  and divide once by W_a = sum_b w_ab.

Layouts on-chip (SBUF):
  x_aT  [f, s]   (PE-transposed at prologue; f on partitions in 128-tiles)
  Q^T   [j, s]   from Wq^T-stationary matmul (j = inner dim)
  K^T   [j, s]
  V     [s, j]   natural
  E^T   [sk, sq] = exp(K^T.T @ Q^T) so that E^T is the *moving* operand of
                   EV^T = V.T @ E^T  (V tiles stationary)
  w_ab via ones-column matmul: ones[128,1].T @ E^T accumulated in a PSUM row.

All matmuls run in float32r (fp32 bits, reduced-precision PE mode, 4x the
fp32 matmul rate at moving-dim >= 256).
"""

import math
import sys

sys.path.insert(0, "/opt/trn_rl_repo")

import numpy as np

GF = (512, 256, 512, 256)
NG = 4
B, S = 8, 1024
ST = 8          # 128-row tiles of S
SH = 2          # 512-col halves of S
NT = tuple(g // 128 for g in GF)
INNER = [[min(GF[a], GF[b]) // 2 for b in range(NG)] for a in range(NG)]

_DT_MM = None   # set lazily (mybir import)


def build_nc(mm_rdtype=True):
    import concourse.tile as tile
    import concourse.mybir as mybir
    from concourse import bacc

    f32 = mybir.dt.float32
    mmdt = mybir.dt.float32r if mm_rdtype else mybir.dt.float32
    Exp = mybir.ActivationFunctionType.Exp

    def mm(ap):
        return ap

    nc = bacc.Bacc(None)

    x_d = [nc.declare_dram_parameter(f"x{a}", [S, GF[a]], f32, isOutput=False)
           for a in range(NG)]
    wq_d = {}
    wk_d = {}
    wv_d = {}
    wo_d = {}
    for a in range(NG):
        for b in range(NG):
            I = INNER[a][b]
            wq_d[a, b] = nc.declare_dram_parameter(f"wq{a}{b}", [GF[a], I], mmdt, isOutput=False)
            wk_d[a, b] = nc.declare_dram_parameter(f"wk{a}{b}", [GF[b], I], mmdt, isOutput=False)
            wv_d[a, b] = nc.declare_dram_parameter(f"wv{a}{b}", [GF[b], I], mmdt, isOutput=False)
            wo_d[a, b] = nc.declare_dram_parameter(f"wo{a}{b}", [I, GF[a]], mmdt, isOutput=False)
    out_d = [nc.declare_dram_parameter(f"out{a}", [S, GF[a]], f32, isOutput=True)
             for a in range(NG)]

    from concourse.masks import make_identity

    with tile.TileContext(nc) as tc:
        from contextlib import ExitStack
        with ExitStack() as ctx:
            const_pool = ctx.enter_context(tc.tile_pool(name="const", bufs=1))
            identity = const_pool.tile([128, 128], f32, tag="ident", name="ident")
            make_identity(nc, identity[:, :])
            ones_f = const_pool.tile([128, 1], f32, tag="ones_f", name="ones_f")
            nc.gpsimd.memset(ones_f[:, :], 1.0)
            ones = const_pool.tile([128, 1], mmdt, tag="ones", name="ones")
            nc.vector.tensor_copy(ones[:, :], ones_f[:, :])

            xT_pool = ctx.enter_context(tc.tile_pool(name="xT", bufs=1))
            xT = [xT_pool.tile([128, NT[a] * S], mmdt, tag=f"xT{a}", name=f"xT{a}") for a in range(NG)]

            # ---- prologue: load x and transpose to [f, s] ----
            with tc.tile_pool(name="xnat", bufs=1) as xnat_pool, \
                 tc.tile_pool(name="ps_t", bufs=4, space="PSUM") as ps_t:
                for a in range(NG):
                    for st in range(ST):
                        xn = xnat_pool.tile([128, GF[a]], f32, tag=f"xn{a}_{st}", name=f"xn{a}_{st}")
                        nc.sync.dma_start(xn[:, :], x_d[a][st * 128:(st + 1) * 128, :])
                        for ft in range(NT[a]):
                            pt = ps_t.tile([128, 128], f32, tag="pt", name="pt")
                            nc.tensor.transpose(pt[:, :], xn[:, ft * 128:(ft + 1) * 128], identity[:, :])
                            nc.vector.tensor_copy(
                                xT[a][:, ft * S + st * 128: ft * S + (st + 1) * 128], pt[:, :])

            # ---- main pools ----
            w_pool = ctx.enter_context(tc.tile_pool(name="wts", bufs=2))
            qk_pool = ctx.enter_context(tc.tile_pool(name="qk", bufs=2))
            v_pool = ctx.enter_context(tc.tile_pool(name="v", bufs=3))
            e_pool = ctx.enter_context(tc.tile_pool(name="e", bufs=6))
            o_pool = ctx.enter_context(tc.tile_pool(name="o", bufs=3))
            ps_w = ctx.enter_context(tc.tile_pool(name="ps_w", bufs=2, space="PSUM"))
            ps_l = ctx.enter_context(tc.tile_pool(name="ps_l", bufs=2, space="PSUM"))
            ps_ev = ctx.enter_context(tc.tile_pool(name="ps_ev", bufs=2, space="PSUM"))
            ps_qz = ctx.enter_context(tc.tile_pool(name="ps_qz", bufs=2, space="PSUM"))

            for a in range(NG):
                with tc.tile_pool(name=f"row{a}", bufs=1) as row_pool:
                    w_ps = [ps_w.tile([1, 512], f32, tag="w", name="w_ps") for _ in range(SH)]
                    evt = {}      # (b, dt) -> [128, S] tile, d-block dt of EV^T
                    wo_t = {}     # b -> [128, IT*GF[a]] tile
                    for b in range(NG):
                        I = INNER[a][b]
                        IT = I // 128
                        NTa, NTb = NT[a], NT[b]
                        scale = 1.0 / math.sqrt(I)

                        # -- weights in --
                        wq_t = w_pool.tile([128, NTa * I], mmdt, tag="wq", name="wq_t")
                        wk_t = w_pool.tile([128, NTb * I], mmdt, tag="wk", name="wk_t")
                        wv_t = w_pool.tile([128, NTb * I], mmdt, tag="wv", name="wv_t")
                        wo_t[b] = row_pool.tile([128, IT * GF[a]], mmdt, tag=f"wo{b}", name=f"wo{b}")
                        for ft in range(NTa):
                            nc.sync.dma_start(wq_t[:, ft * I:(ft + 1) * I],
                                              wq_d[a, b][ft * 128:(ft + 1) * 128, :])
                        for ft in range(NTb):
                            nc.sync.dma_start(wk_t[:, ft * I:(ft + 1) * I],
                                              wk_d[a, b][ft * 128:(ft + 1) * 128, :])
                            nc.sync.dma_start(wv_t[:, ft * I:(ft + 1) * I],
                                              wv_d[a, b][ft * 128:(ft + 1) * 128, :])
                        for dt in range(IT):
                            nc.sync.dma_start(wo_t[b][:, dt * GF[a]:(dt + 1) * GF[a]],
                                              wo_d[a, b][dt * 128:(dt + 1) * 128, :])

                        # -- Q^T = Wq.T @ x_aT ; K^T = Wk.T @ x_bT --
                        qt = qk_pool.tile([128, IT * S], mmdt, tag="qt", name="qt")
                        kt = qk_pool.tile([128, IT * S], mmdt, tag="kt", name="kt")
                        for (dst, wt, xa, nta) in ((qt, wq_t, xT[a], NTa), (kt, wk_t, xT[b], NTb)):
                            for jt in range(IT):
                                for sh in range(SH):
                                    q_ps = ps_qz.tile([128, 512], f32, tag="qz", name="qz_ps")
                                    for ft in range(nta):
                                        nc.tensor.matmul(
                                            q_ps[:, :],
                                            mm(wt[:, ft * I + jt * 128: ft * I + (jt + 1) * 128]),
                                            mm(xa[:, ft * S + sh * 512: ft * S + (sh + 1) * 512]),
                                            start=(ft == 0), stop=(ft == nta - 1))
                                    nc.vector.tensor_copy(
                                        dst[:, jt * S + sh * 512: jt * S + (sh + 1) * 512], q_ps[:, :])

                        # -- V = x_b @ Wv  (natural [s, j]) --
                        vt = v_pool.tile([128, ST * I], mmdt, tag="vt", name="vt")
                        for st in range(ST):
                            v_ps = ps_qz.tile([128, I], f32, tag="qz", name="qz_ps")
                            for ft in range(NTb):
                                nc.tensor.matmul(
                                    v_ps[:, :],
                                    mm(xT[b][:, ft * S + st * 128: ft * S + (st + 1) * 128]),
                                    mm(wv_t[:, ft * I:(ft + 1) * I]),
                                    start=(ft == 0), stop=(ft == NTb - 1))
                            nc.vector.tensor_copy(vt[:, st * I:(st + 1) * I], v_ps[:, :])

                        # -- attention: E^T tiles, EV^T accumulation, w row --
                        for dt in range(IT):
                            evt[b, dt] = row_pool.tile([128, S], mmdt, tag=f"evt{b}_{dt}", name=f"evt{b}_{dt}")
                        for sh in range(SH):
                            ev_ps = [ps_ev.tile([128, 512], f32, tag="ev", name="ev_ps") for _ in range(IT)]
                            for sk in range(ST):
                                l_ps = ps_l.tile([128, 512], f32, tag="l", name="l_ps")
                                for jt in range(IT):
                                    nc.tensor.matmul(
                                        l_ps[:, :],
                                        mm(kt[:, jt * S + sk * 128: jt * S + (sk + 1) * 128]),
                                        mm(qt[:, jt * S + sh * 512: jt * S + (sh + 1) * 512]),
                                        start=(jt == 0), stop=(jt == IT - 1))
                                et = e_pool.tile([128, 512], mmdt, tag="et", name="et")
                                nc.scalar.activation(et[:, :], l_ps[:, :], Exp, scale=scale)
                                for dt in range(IT):
                                    nc.tensor.matmul(
                                        ev_ps[dt][:, :],
                                        mm(vt[:, sk * I + dt * 128: sk * I + (dt + 1) * 128]),
                                        mm(et[:, :]),
                                        start=(sk == 0), stop=(sk == ST - 1))
                                nc.tensor.matmul(
                                    w_ps[sh][:, :], mm(ones[:, 0:1]), mm(et[:, :]),
                                    start=(b == 0 and sk == 0),
                                    stop=(b == NG - 1 and sk == ST - 1))
                            for dt in range(IT):
                                nc.vector.tensor_copy(
                                    evt[b, dt][:, sh * 512:(sh + 1) * 512], ev_ps[dt][:, :])

                    # -- row tail: recip weights, O-projection, divide, out --
                    recip = o_pool.tile([1, S], f32, tag="recip", name="recip")
                    for sh in range(SH):
                        nc.vector.reciprocal(recip[0:1, sh * 512:(sh + 1) * 512], w_ps[sh][:, :])
                    wcol = o_pool.tile([128, ST], f32, tag="wcol", name="wcol")
                    for st in range(ST):
                        nc.sync.dma_start(wcol[:, st:st + 1],
                                          recip[0:1, st * 128:(st + 1) * 128])

                    for st in range(ST):
                        z_ps = ps_qz.tile([128, GF[a]], f32, tag="qz", name="qz_ps")
                        steps = [(b, dt) for b in range(NG) for dt in range(INNER[a][b] // 128)]
                        for i, (b, dt) in enumerate(steps):
                            nc.tensor.matmul(
                                z_ps[:, :],
                                mm(evt[b, dt][:, st * 128:(st + 1) * 128]),
                                mm(wo_t[b][:, dt * GF[a]:(dt + 1) * GF[a]]),
                                start=(i == 0), stop=(i == len(steps) - 1))
                        out_sb = o_pool.tile([128, GF[a]], f32, tag="out", name="out_sb")
                        nc.vector.tensor_scalar_mul(out_sb[:, :], z_ps[:, :], wcol[:, st:st + 1])
                        nc.sync.dma_start(out_d[a][st * 128:(st + 1) * 128, :], out_sb[:, :])

    nc.finalize()
    return nc


_CACHED_NC = None


def kernel(words0, words1, words2, words3, params):
    from concourse.bass_utils import run_bass_kernel_spmd

    global _CACHED_NC
    if _CACHED_NC is None:
        _CACHED_NC = build_nc()
    nc = _CACHED_NC

    words = [np.ascontiguousarray(np.asarray(w, dtype=np.float32))
             for w in (words0, words1, words2, words3)]
    wmap = {}
    for a in range(NG):
        for b in range(NG):
            p = params[a][b]
            wmap[f"wq{a}{b}"] = np.ascontiguousarray(np.asarray(p["Wq"], dtype=np.float32))
            wmap[f"wk{a}{b}"] = np.ascontiguousarray(np.asarray(p["Wk"], dtype=np.float32))
            wmap[f"wv{a}{b}"] = np.ascontiguousarray(np.asarray(p["Wv"], dtype=np.float32))
            wmap[f"wo{a}{b}"] = np.ascontiguousarray(np.asarray(p["Wo"], dtype=np.float32))

    core_ids = list(range(B))
    in_maps = []
    for i in core_ids:
        m = dict(wmap)
        for a in range(NG):
            m[f"x{a}"] = words[a][i]
        in_maps.append(m)

    res = run_bass_kernel_spmd(nc, in_maps, core_ids)
    outs = []
    for a in range(NG):
        outs.append(np.stack([res.results[i][f"out{a}"] for i in range(B)], axis=0))
    return tuple(outs)
